# revision 40
# baseline (speedup 1.0000x reference)
"""DividedAttentionSublayer on 8 TRN2 NeuronCores.

Sharding: data-parallel over batch (B=8 -> 1 batch element per core),
weights / pos_emb replicated. Per core the attention runs in a
transposed layout (k on partitions, q on free dim). All matmul inputs
are bf16 (host-converted). The relative-position band is fused directly
into the logits PSUM: a skewed-stride DRAM re-read (rel-shift trick)
produces the band in [q, k] order, and a regular matmul against the
identity both transposes it and accumulates it into the logits psum
before a single exp. Clamped tails (|k-q| > 128) enter through two
augmented q rows whose weights (ep2^T Wq / scale) are folded into the
projection host-side; the V bias is folded into the output bias
(bo' = bo + Wo @ bv).
"""
import sys

sys.path.insert(0, "/opt/trn_rl_repo")

import numpy as np
import ml_dtypes
import concourse.bass as bass
import concourse.mybir as mybir
from concourse import bacc
from concourse.tile import TileContext
from concourse.bass import AP
from concourse.bass_utils import run_bass_kernel_spmd

F32 = mybir.dt.float32
BF16 = mybir.dt.bfloat16
EXP = mybir.ActivationFunctionType.Exp

B, L, D = 8, 1024, 1024
H, DH = 16, 64
NT = L // 128
SCALE = float(np.sqrt(D / H))
JW = 257           # 2*128 + 1 relative positions
PW = 513           # per-q-tile dpad chunk: 128 pad | 257 mid | 128 pad
PXW = NT * PW      # 4104
BW = 384           # gathered band width per q-tile (3 k-tiles)

_NC = None


def _build():
    nc = bacc.Bacc(None, target_bir_lowering=False)

    xq = nc.dram_tensor("xq", [D, L], BF16, kind="ExternalInput")
    xk = nc.dram_tensor("xk", [D, L], BF16, kind="ExternalInput")
    xv = nc.dram_tensor("xv", [D, L], BF16, kind="ExternalInput")
    wq = nc.dram_tensor("wq", [D, D], BF16, kind="ExternalInput")
    wk = nc.dram_tensor("wk", [D, D], BF16, kind="ExternalInput")
    wv = nc.dram_tensor("wv", [D, D], BF16, kind="ExternalInput")
    wo = nc.dram_tensor("wo", [D, D], BF16, kind="ExternalInput")
    wqa = nc.dram_tensor("wqa", [D, 2 * H], BF16, kind="ExternalInput")
    bqc = nc.dram_tensor("bqc", [128, NT], F32, kind="ExternalInput")
    bkc = nc.dram_tensor("bkc", [128, NT], F32, kind="ExternalInput")
    boc = nc.dram_tensor("boc", [128, NT], F32, kind="ExternalInput")
    bqac = nc.dram_tensor("bqac", [2 * H, 1], F32, kind="ExternalInput")
    mkb = nc.dram_tensor("mkb", [128, NT], F32, kind="ExternalInput")
    ept = nc.dram_tensor("ept", [DH, JW], BF16, kind="ExternalInput")
    idn = nc.dram_tensor("idn", [128, 128], BF16, kind="ExternalInput")
    outt = nc.dram_tensor("outt", [D, L], BF16, kind="ExternalOutput")

    rw = lambda t: t[:].rearrange("(c p) l -> p c l", p=128)

    with TileContext(nc) as tc:
        with (
            tc.tile_pool(name="persist", bufs=1) as pp,
            tc.tile_pool(name="qth", bufs=16) as pool_qth,
            tc.tile_pool(name="kth", bufs=16) as pool_kth,
            tc.tile_pool(name="vsb", bufs=8) as pool_v,
            tc.tile_pool(name="ct", bufs=8) as pool_ct,
            tc.tile_pool(name="pex", bufs=2) as pool_pex,
            tc.tile_pool(name="gts", bufs=3) as pool_g,
            tc.tile_pool(name="psP", bufs=2, space="PSUM") as pool_psP,
            tc.tile_pool(name="dram", bufs=5, space="DRAM") as pool_d,
        ):
            bq_sb = pp.tile([128, NT], F32, tag="bq")
            bk_sb = pp.tile([128, NT], F32, tag="bk")
            bo_sb = pp.tile([128, NT], F32, tag="bo")
            bqa_sb = pp.tile([2 * H, 1], F32, tag="bqa")
            mk_sb = pp.tile([128, NT], F32, tag="mk")
            ept_b = pp.tile([DH, JW], BF16, tag="eptb")
            ident = pp.tile([128, 128], BF16, tag="ident")
            wqa_sb = pp.tile([128, NT * 2 * H], BF16, tag="wqa")
            wo_first = pp.tile([128, NT * 512], BF16, tag="wof")
            qth = [pool_qth.tile([66, L], BF16, tag="qth", name=f"qth{i}") for i in range(H)]
            kth = [pool_kth.tile([66, L], BF16, tag="kth", name=f"kth{i}") for i in range(H)]
            v_sb = [pool_v.tile([128, H * 65], BF16, tag="v", name=f"vsb{i}") for i in range(NT)]
            ct = [pool_ct.tile([128, L], BF16, tag="ct", name=f"ct{i}") for i in range(NT)]

            for h in range(H):
                nc.gpsimd.memset(kth[h][64:66, :], 1.0)
            for lt in range(NT):
                nc.gpsimd.memset(
                    v_sb[lt][:].rearrange("p (h c) -> p h c", c=65)[:, :, 64:65], 1.0
                )

            gtiles = {}

            pex_cur = {}

            def band_prep_step(h, m):
                if m == 0:
                    pex_cur[h] = pool_pex.tile([128, PXW], BF16, tag="pex",
                                               name=f"pex{h}")
                pexh = pex_cur[h]
                pp_ps = pool_psP.tile([128, JW], F32, tag="pps")
                nc.tensor.matmul(
                    pp_ps[:], qth[h][0:64, 128 * m : 128 * m + 128], ept_b[:],
                    start=True, stop=True,
                )
                nc.vector.tensor_copy(
                    pexh[:, PW * m + 128 : PW * m + 128 + JW], pp_ps[:]
                )
                # both clamp pads in one op: cols [0,128) <- P col 0,
                # cols [385,513) <- P col 256
                nc.gpsimd.tensor_copy(
                    AP(pexh.tensor, pexh.offset + PW * m,
                       [[PXW, 128], [385, 2], [1, 128]]),
                    AP(pexh.tensor, pexh.offset + PW * m + 128,
                       [[PXW, 128], [256, 2], [0, 128]]),
                )

            def band_finish(h):
                pexh = pex_cur.pop(h)
                dpad = pool_d.tile([128, PXW], BF16, tag="dpad")
                nc.sync.dma_start(dpad[:], pexh[:])
                gh = pool_g.tile([128, NT * BW], BF16, tag="g", name=f"g{h}")
                nc.sync.dma_start(
                    gh[:].rearrange("p (m j) -> p m j", j=BW),
                    AP(dpad.tensor, dpad.offset + 128,
                       [[PXW - 1, 128], [PW, NT], [1, BW]]),
                )
                gtiles[h] = gh

            def band_prep(h):
                for m in range(NT):
                    band_prep_step(h, m)
                band_finish(h)

            # =========== Q/K/V projections (bf16) ===========
            with (
                tc.tile_pool(name="xin", bufs=2) as pool_x,
                tc.tile_pool(name="win", bufs=2) as pool_w,
                tc.tile_pool(name="pps", bufs=4, space="PSUM") as pool_ps,
                tc.tile_pool(name="pps2", bufs=2, space="PSUM") as pool_ps2,
            ):
                x_sb = {}
                w_sb = {}

                def load_x(name, src):
                    t = pool_x.tile([128, NT * L], BF16, tag="x", name=f"x_{name}")
                    tr = t[:].rearrange("p (c l) -> p c l", l=L)
                    sr = rw(src)
                    for half in range(2):
                        nc.sync.dma_start(
                            tr[:, :, 512 * half : 512 * half + 512],
                            sr[:, :, 512 * half : 512 * half + 512],
                        )
                    x_sb[name] = t

                def load_w(name, src, split_first=False):
                    t = pool_w.tile([128, NT * D], BF16, tag="w", name=f"w_{name}")
                    tr = t[:].rearrange("p (c l) -> p c l", l=D)
                    sr = rw(src)
                    if split_first:
                        nc.sync.dma_start(tr[:, :, 0:128], sr[:, :, 0:128])
                        w_sb[name] = t
                        return
                    nc.sync.dma_start(tr[:, :, 0:512], sr[:, :, 0:512])
                    nc.sync.dma_start(tr[:, :, 512:D], sr[:, :, 512:D])
                    w_sb[name] = t

                load_w("q", wq, split_first=True)
                load_x("q", xq)
                nc.sync.dma_start(bq_sb[:], bqc[:])
                wqt = w_sb["q"][:].rearrange("p (c l) -> p c l", l=D)
                wqs = rw(wq)
                nc.sync.dma_start(wqt[:, :, 128:512], wqs[:, :, 128:512])
                nc.sync.dma_start(wqt[:, :, 512:D], wqs[:, :, 512:D])
                # consts
                nc.sync.dma_start(
                    wqa_sb[:].rearrange("p (c m) -> p c m", m=2 * H),
                    wqa[:].rearrange("(c p) m -> p c m", p=128),
                )
                for t, src in ((bk_sb, bkc), (bo_sb, boc),
                               (bqa_sb, bqac), (mk_sb, mkb), (ept_b, ept),
                               (ident, idn)):
                    nc.sync.dma_start(t[:], src[:])
                load_x("k", xk)
                load_w("k", wk)

                def proj_qk(xname, wname, dst, bcol):
                    xt = x_sb[xname][:].rearrange("p (c l) -> p c l", l=L)
                    wt = w_sb[wname][:].rearrange("p (c l) -> p c l", l=D)
                    for i in range(NT):
                        for lh in range(2):
                            ps = pool_ps.tile([128, 512], F32, tag="ps")
                            for c in range(NT):
                                nc.tensor.matmul(
                                    ps[:],
                                    wt[:, c, 128 * i : 128 * i + 128],
                                    xt[:, c, 512 * lh : 512 * lh + 512],
                                    start=(c == 0),
                                    stop=(c == NT - 1),
                                )
                            for half in range(2):
                                hh = 2 * i + half
                                nc.vector.tensor_scalar_add(
                                    dst[hh][0:64, 512 * lh : 512 * lh + 512],
                                    ps[64 * half : 64 * half + 64, :],
                                    bcol[64 * half : 64 * half + 64, i : i + 1],
                                )

                proj_qk("q", "q", qth, bq_sb)
                # augmented q rows for all heads: [32, 512] psum per half
                xt = x_sb["q"][:].rearrange("p (c l) -> p c l", l=L)
                wa = wqa_sb[:].rearrange("p (c m) -> p c m", m=2 * H)
                qaug = pp.tile([2 * H, L], BF16, tag="qaug")
                for lh in range(2):
                    ps2 = pool_ps2.tile([2 * H, 512], F32, tag="ps2")
                    for c in range(NT):
                        nc.tensor.matmul(
                            ps2[:], wa[:, c, :],
                            xt[:, c, 512 * lh : 512 * lh + 512],
                            start=(c == 0), stop=(c == NT - 1),
                        )
                    nc.vector.tensor_scalar_add(
                        qaug[:, 512 * lh : 512 * lh + 512], ps2[:], bqa_sb[:, 0:1]
                    )
                for h in range(H):
                    nc.gpsimd.dma_start(
                        qth[h][64:66, :], qaug[2 * h : 2 * h + 2, :]
                    )

                load_x("v", xv)
                load_w("v", wv)
                for h in range(2):
                    band_prep(h)
                proj_qk("k", "k", kth, bk_sb)

                # ---- V projection (natural layout, no bias) ----
                xt = x_sb["v"][:].rearrange("p (c l) -> p c l", l=L)
                wt = w_sb["v"][:].rearrange("p (c l) -> p c l", l=D)
                for dh_ in range(2):
                    for lt in range(NT):
                        ps = pool_ps.tile([128, 512], F32, tag="ps")
                        for c in range(NT):
                            nc.tensor.matmul(
                                ps[:],
                                xt[:, c, 128 * lt : 128 * lt + 128],
                                wt[:, c, 512 * dh_ : 512 * dh_ + 512],
                                start=(c == 0),
                                stop=(c == NT - 1),
                            )
                        nc.vector.tensor_copy(
                            v_sb[lt][:].rearrange("p (h c) -> p h c", c=65)[
                                :, 8 * dh_ : 8 * dh_ + 8, 0:64
                            ],
                            ps[:].rearrange("p (a b) -> p a b", a=8),
                        )

            # =========== attention ===========
            with (
                tc.tile_pool(name="attn", bufs=10) as pool_attn,
                tc.tile_pool(name="scratch", bufs=2) as pool_s,
                tc.tile_pool(name="psL", bufs=2, space="PSUM") as pool_psL,
                tc.tile_pool(name="psV", bufs=2, space="PSUM") as pool_psV,
            ):
                LOOK = 2
                opre = []
                for h in range(H):
                    q = qth[h]
                    k = kth[h]
                    gh = gtiles.pop(h)
                    gr = gh[:].rearrange("p (m j) -> p m j", j=BW)

                    attn = []
                    pavs = [pool_psV.tile([65, 512], F32, tag="pav",
                                           name=f"pav{lh}") for lh in range(2)]
                    for n in range(NT):
                        pl = pool_psL.tile([128, L], F32, tag="pl")
                        b0, b1 = max(n - 1, 0), min(n + 2, NT)
                        # far spans with clamp-tail aug rows (no g dependence)
                        spans = []
                        if 128 * (n + 2) < L:
                            spans.append((128 * (n + 2), L, 65))
                        if n - 1 > 0:
                            spans.append((0, 128 * (n - 1), 66))
                        for s0, s1, kk in spans:
                            c0 = s0
                            while c0 < s1:
                                c1 = min(s1, (c0 // 512 + 1) * 512)
                                nc.tensor.matmul(
                                    pl[:, c0:c1],
                                    k[0:kk, 128 * n : 128 * n + 128],
                                    q[0:kk, c0:c1],
                                    start=True, stop=True,
                                    skip_group_check=True,
                                )
                                c0 = c1
                        # band: QK (start) in <=512 chunks, then the
                        # transposed pos band accumulated via identity matmul
                        c0 = 128 * b0
                        while c0 < 128 * b1:
                            c1 = min(128 * b1, (c0 // 512 + 1) * 512)
                            nc.tensor.matmul(
                                pl[:, c0:c1],
                                k[0:64, 128 * n : 128 * n + 128],
                                q[0:64, c0:c1],
                                start=True, stop=False,
                                skip_group_check=True,
                            )
                            c0 = c1
                        for m in range(b0, b1):
                            jb = n - m + 1
                            nc.tensor.matmul(
                                pl[:, 128 * m : 128 * m + 128],
                                gr[:, m, 128 * jb : 128 * jb + 128],
                                ident[:],
                                start=False, stop=True,
                                skip_group_check=True,
                            )
                        at = pool_attn.tile([128, L], BF16, tag="at")
                        nc.scalar.activation(at[:], pl[:], EXP, bias=mk_sb[:, n : n + 1])
                        attn.append(at)
                        if h + LOOK < H:
                            band_prep_step(h + LOOK, n)

                    if h + LOOK < H:
                        band_finish(h + LOOK)
                    if h == 9:
                        nc.sync.dma_start(
                            wo_first[:].rearrange("p (c l) -> p c l", l=512),
                            rw(wo)[:, :, 0:512],
                        )
                    if h >= 14:
                        lh0 = h - 14
                        pso = pool_psP.tile([128, 512], F32, tag="pps",
                                            name=f"opre{lh0}")
                        wf0 = wo_first[:].rearrange("p (c l) -> p c l", l=512)
                        for c in range(NT - 1):
                            nc.tensor.matmul(
                                pso[:],
                                wf0[:, c, 0:128],
                                ct[c][:, 512 * lh0 : 512 * lh0 + 512],
                                start=(c == 0), stop=False,
                                skip_group_check=True,
                            )
                        opre.append(pso)
                    for lh in range(2):
                        for n in range(NT - 1):
                            nc.tensor.matmul(
                                pavs[lh][:],
                                v_sb[n][:, 65 * h : 65 * h + 65],
                                attn[n][:, 512 * lh : 512 * lh + 512],
                                start=(n == 0),
                                stop=False,
                            )
                    for lh in range(2):
                        nc.tensor.matmul(
                            pavs[lh][:],
                            v_sb[NT - 1][:, 65 * h : 65 * h + 65],
                            attn[NT - 1][:, 512 * lh : 512 * lh + 512],
                            start=False,
                            stop=True,
                        )
                    for lh in range(2):
                        pav = pavs[lh]
                        rec = pool_s.tile([1, 512], F32, tag="rec")
                        nc.vector.reciprocal(rec[:], pav[64:65, :])
                        pbm = pool_s.tile([64, 512], F32, tag="pbm")
                        nc.gpsimd.partition_broadcast(pbm[:], rec[:])
                        nc.vector.tensor_mul(
                            ct[h // 2][
                                64 * (h % 2) : 64 * (h % 2) + 64, 512 * lh : 512 * lh + 512
                            ],
                            pav[0:64, :],
                            pbm[:],
                        )

            # =========== output projection ===========
            with (
                tc.tile_pool(name="wout", bufs=1) as pool_wo,
                tc.tile_pool(name="oo", bufs=4) as pool_o,
                tc.tile_pool(name="ops", bufs=4, space="PSUM") as pool_ops,
            ):
                wot = pool_wo.tile([128, NT * D], BF16, tag="wo")
                wr = wot[:].rearrange("p (c l) -> p c l", l=D)
                wf = wo_first[:].rearrange("p (c l) -> p c l", l=512)
                sr = rw(wo)
                nc.sync.dma_start(wr[:, :, 512:768], sr[:, :, 512:768])
                nc.sync.dma_start(wr[:, :, 768:D], sr[:, :, 768:D])
                for i in range(NT):
                    ot = pool_o.tile([128, L], BF16, tag="ot")
                    for lh in range(2):
                        if i == 0:
                            ps = opre[lh]
                            nc.tensor.matmul(
                                ps[:],
                                wf[:, NT - 1, 0:128],
                                ct[NT - 1][:, 512 * lh : 512 * lh + 512],
                                start=False, stop=True,
                                skip_group_check=True,
                            )
                        else:
                            ps = pool_ops.tile([128, 512], F32, tag="ps")
                            wsrc, wcol = (wf, 128 * i) if i < 4 else (wr, 128 * i)
                            for c in range(NT):
                                nc.tensor.matmul(
                                    ps[:],
                                    wsrc[:, c, wcol : wcol + 128],
                                    ct[c][:, 512 * lh : 512 * lh + 512],
                                    start=(c == 0),
                                    stop=(c == NT - 1),
                                )
                        nc.vector.tensor_scalar_add(
                            ot[:, 512 * lh : 512 * lh + 512], ps[:], bo_sb[:, i : i + 1]
                        )
                    if i == NT - 1:
                        for lh in range(2):
                            nc.sync.dma_start(
                                outt[128 * i : 128 * i + 128, 512 * lh : 512 * lh + 512],
                                ot[:, 512 * lh : 512 * lh + 512],
                            )
                    else:
                        nc.sync.dma_start(outt[128 * i : 128 * i + 128, :], ot[:])

    nc.compile()
    return nc


def _get_nc():
    global _NC
    if _NC is None:
        _NC = _build()
    return _NC


def _prep_shared(Wq, bq, Wk, bk, Wv, bv, Wo, bo, pos_emb):
    bf = ml_dtypes.bfloat16
    Wq = np.asarray(Wq, np.float32)
    Wk = np.asarray(Wk, np.float32)
    Wv = np.asarray(Wv, np.float32)
    Wo = np.asarray(Wo, np.float32)
    bq = np.asarray(bq, np.float32)
    bk = np.asarray(bk, np.float32)
    bv = np.asarray(bv, np.float32)
    bo = np.asarray(bo, np.float32)
    ep = np.asarray(pos_emb, np.float32)

    wq_arr = np.ascontiguousarray(Wq.T / SCALE)
    wk_arr = np.ascontiguousarray(Wk.T)
    wv_arr = np.ascontiguousarray(Wv.T)
    wo_arr = np.ascontiguousarray(Wo.T)

    ep2 = np.stack([ep[0], ep[2 * 128] - ep[0]], axis=1)       # [64, 2]
    Wq3 = (Wq / SCALE).reshape(H, DH, D)
    # wqa[:, 2h+j] = sum_r ep2[r, j] * Wq3[h, r, :]
    wqa_arr = np.ascontiguousarray(
        np.einsum("rj,hrd->dhj", ep2, Wq3).reshape(D, 2 * H)
    )
    bq3 = (bq / SCALE).reshape(H, DH)
    bqa_arr = np.ascontiguousarray(np.einsum("rj,hr->hj", ep2, bq3).reshape(2 * H, 1))

    bo2 = bo + Wo @ bv
    return {
        "wq": wq_arr.astype(bf), "wk": wk_arr.astype(bf),
        "wv": wv_arr.astype(bf), "wo": wo_arr.astype(bf),
        "wqa": wqa_arr.astype(bf),
        "bqc": np.ascontiguousarray((bq / SCALE).reshape(NT, 128).T),
        "bkc": np.ascontiguousarray(bk.reshape(NT, 128).T),
        "boc": np.ascontiguousarray(bo2.reshape(NT, 128).T),
        "bqac": np.ascontiguousarray(bqa_arr),
        "ept": np.ascontiguousarray(ep.T).astype(bf),
        "idn": np.eye(128, dtype=np.float32).astype(bf),
    }


def _per_core_inputs(shared, inputs, b):
    bf = ml_dtypes.bfloat16
    mrow = np.asarray(inputs["mask"])[b].reshape(L).astype(bool)
    m = dict(shared)
    m["xq"] = np.ascontiguousarray(np.asarray(inputs["x_q"], np.float32)[b].T).astype(bf)
    m["xk"] = np.ascontiguousarray(np.asarray(inputs["x_k"], np.float32)[b].T).astype(bf)
    m["xv"] = np.ascontiguousarray(np.asarray(inputs["x_v"], np.float32)[b].T).astype(bf)
    m["mkb"] = np.ascontiguousarray(
        np.where(mrow, np.float32(-1e30), np.float32(0.0)).reshape(NT, 128).T
    )
    return m


def kernel(x_q, x_k, x_v, mask, Wq, bq, Wk, bk, Wv, bv, Wo, bo, pos_emb):
    nc = _get_nc()
    shared = _prep_shared(Wq, bq, Wk, bk, Wv, bv, Wo, bo, pos_emb)
    inputs = {"x_q": x_q, "x_k": x_k, "x_v": x_v, "mask": mask}
    in_maps = [_per_core_inputs(shared, inputs, b) for b in range(B)]
    res = run_bass_kernel_spmd(nc, in_maps, core_ids=list(range(B)))
    out = np.empty((B, L, D), np.float32)
    for b in range(B):
        out[b] = np.asarray(res.results[b]["outt"], np.float32).T
    return out


# revision 41
# speedup vs baseline: 1.0017x; 1.0017x over previous
"""DividedAttentionSublayer on 8 TRN2 NeuronCores.

Sharding: data-parallel over batch (B=8 -> 1 batch element per core),
weights / pos_emb replicated. Per core the attention runs in a
transposed layout (k on partitions, q on free dim). All matmul inputs
are bf16 (host-converted). The relative-position band is fused directly
into the logits PSUM: a skewed-stride DRAM re-read (rel-shift trick)
produces the band in [q, k] order, and a regular matmul against the
identity both transposes it and accumulates it into the logits psum
before a single exp. Clamped tails (|k-q| > 128) enter through two
augmented q rows whose weights (ep2^T Wq / scale) are folded into the
projection host-side; the V bias is folded into the output bias
(bo' = bo + Wo @ bv).
"""
import sys

sys.path.insert(0, "/opt/trn_rl_repo")

import numpy as np
import ml_dtypes
import concourse.bass as bass
import concourse.mybir as mybir
from concourse import bacc
from concourse.tile import TileContext
from concourse.bass import AP
from concourse.bass_utils import run_bass_kernel_spmd

F32 = mybir.dt.float32
BF16 = mybir.dt.bfloat16
EXP = mybir.ActivationFunctionType.Exp

B, L, D = 8, 1024, 1024
H, DH = 16, 64
NT = L // 128
SCALE = float(np.sqrt(D / H))
JW = 257           # 2*128 + 1 relative positions
PW = 513           # per-q-tile dpad chunk: 128 pad | 257 mid | 128 pad
PXW = NT * PW      # 4104
BW = 384           # gathered band width per q-tile (3 k-tiles)

_NC = None


def _build():
    nc = bacc.Bacc(None, target_bir_lowering=False)

    xq = nc.dram_tensor("xq", [D, L], BF16, kind="ExternalInput")
    xk = nc.dram_tensor("xk", [D, L], BF16, kind="ExternalInput")
    xv = nc.dram_tensor("xv", [D, L], BF16, kind="ExternalInput")
    wq = nc.dram_tensor("wq", [D, D], BF16, kind="ExternalInput")
    wk = nc.dram_tensor("wk", [D, D], BF16, kind="ExternalInput")
    wv = nc.dram_tensor("wv", [D, D], BF16, kind="ExternalInput")
    wo = nc.dram_tensor("wo", [D, D], BF16, kind="ExternalInput")
    wqa = nc.dram_tensor("wqa", [D, 2 * H], BF16, kind="ExternalInput")
    bqc = nc.dram_tensor("bqc", [128, NT], F32, kind="ExternalInput")
    bkc = nc.dram_tensor("bkc", [128, NT], F32, kind="ExternalInput")
    boc = nc.dram_tensor("boc", [128, NT], F32, kind="ExternalInput")
    bqac = nc.dram_tensor("bqac", [2 * H, 1], F32, kind="ExternalInput")
    mkb = nc.dram_tensor("mkb", [128, NT], F32, kind="ExternalInput")
    ept = nc.dram_tensor("ept", [DH, JW], BF16, kind="ExternalInput")
    idn = nc.dram_tensor("idn", [128, 128], BF16, kind="ExternalInput")
    outt = nc.dram_tensor("outt", [D, L], BF16, kind="ExternalOutput")

    rw = lambda t: t[:].rearrange("(c p) l -> p c l", p=128)

    with TileContext(nc) as tc:
        with (
            tc.tile_pool(name="persist", bufs=1) as pp,
            tc.tile_pool(name="qth", bufs=16) as pool_qth,
            tc.tile_pool(name="kth", bufs=16) as pool_kth,
            tc.tile_pool(name="vsb", bufs=8) as pool_v,
            tc.tile_pool(name="ct", bufs=8) as pool_ct,
            tc.tile_pool(name="pex", bufs=2) as pool_pex,
            tc.tile_pool(name="gts", bufs=3) as pool_g,
            tc.tile_pool(name="psP", bufs=2, space="PSUM") as pool_psP,
            tc.tile_pool(name="dram", bufs=5, space="DRAM") as pool_d,
        ):
            bq_sb = pp.tile([128, NT], F32, tag="bq")
            bk_sb = pp.tile([128, NT], F32, tag="bk")
            bo_sb = pp.tile([128, NT], F32, tag="bo")
            bqa_sb = pp.tile([2 * H, 1], F32, tag="bqa")
            mk_sb = pp.tile([128, NT], F32, tag="mk")
            ept_b = pp.tile([DH, JW], BF16, tag="eptb")
            ident = pp.tile([128, 128], BF16, tag="ident")
            wqa_sb = pp.tile([128, NT * 2 * H], BF16, tag="wqa")
            wo_first = pp.tile([128, NT * 512], BF16, tag="wof")
            qth = [pool_qth.tile([66, L], BF16, tag="qth", name=f"qth{i}") for i in range(H)]
            kth = [pool_kth.tile([66, L], BF16, tag="kth", name=f"kth{i}") for i in range(H)]
            v_sb = [pool_v.tile([128, H * 65], BF16, tag="v", name=f"vsb{i}") for i in range(NT)]
            ct = [pool_ct.tile([128, L], BF16, tag="ct", name=f"ct{i}") for i in range(NT)]

            for h in range(H):
                nc.gpsimd.memset(kth[h][64:66, :], 1.0)
            for lt in range(NT):
                nc.gpsimd.memset(
                    v_sb[lt][:].rearrange("p (h c) -> p h c", c=65)[:, :, 64:65], 1.0
                )

            gtiles = {}

            pex_cur = {}

            def band_prep_step(h, m):
                if m == 0:
                    pex_cur[h] = pool_pex.tile([128, PXW], BF16, tag="pex",
                                               name=f"pex{h}")
                pexh = pex_cur[h]
                pp_ps = pool_psP.tile([128, JW], F32, tag="pps")
                nc.tensor.matmul(
                    pp_ps[:], qth[h][0:64, 128 * m : 128 * m + 128], ept_b[:],
                    start=True, stop=True,
                )
                nc.vector.tensor_copy(
                    pexh[:, PW * m + 128 : PW * m + 128 + JW], pp_ps[:]
                )
                # both clamp pads in one op: cols [0,128) <- P col 0,
                # cols [385,513) <- P col 256
                nc.gpsimd.tensor_copy(
                    AP(pexh.tensor, pexh.offset + PW * m,
                       [[PXW, 128], [385, 2], [1, 128]]),
                    AP(pexh.tensor, pexh.offset + PW * m + 128,
                       [[PXW, 128], [256, 2], [0, 128]]),
                )

            def band_finish(h):
                pexh = pex_cur.pop(h)
                dpad = pool_d.tile([128, PXW], BF16, tag="dpad")
                nc.sync.dma_start(dpad[:], pexh[:])
                gh = pool_g.tile([128, NT * BW], BF16, tag="g", name=f"g{h}")
                nc.sync.dma_start(
                    gh[:].rearrange("p (m j) -> p m j", j=BW),
                    AP(dpad.tensor, dpad.offset + 128,
                       [[PXW - 1, 128], [PW, NT], [1, BW]]),
                )
                gtiles[h] = gh

            def band_prep(h):
                for m in range(NT):
                    band_prep_step(h, m)
                band_finish(h)

            # =========== Q/K/V projections (bf16) ===========
            with (
                tc.tile_pool(name="xin", bufs=2) as pool_x,
                tc.tile_pool(name="win", bufs=2) as pool_w,
                tc.tile_pool(name="pps", bufs=4, space="PSUM") as pool_ps,
                tc.tile_pool(name="pps2", bufs=1, space="PSUM") as pool_ps2,
            ):
                x_sb = {}
                w_sb = {}

                def load_x(name, src):
                    t = pool_x.tile([128, NT * L], BF16, tag="x", name=f"x_{name}")
                    tr = t[:].rearrange("p (c l) -> p c l", l=L)
                    sr = rw(src)
                    for half in range(2):
                        nc.sync.dma_start(
                            tr[:, :, 512 * half : 512 * half + 512],
                            sr[:, :, 512 * half : 512 * half + 512],
                        )
                    x_sb[name] = t

                def load_w(name, src, split_first=False):
                    t = pool_w.tile([128, NT * D], BF16, tag="w", name=f"w_{name}")
                    tr = t[:].rearrange("p (c l) -> p c l", l=D)
                    sr = rw(src)
                    if split_first:
                        nc.sync.dma_start(tr[:, :, 0:128], sr[:, :, 0:128])
                        w_sb[name] = t
                        return
                    nc.sync.dma_start(tr[:, :, 0:512], sr[:, :, 0:512])
                    nc.sync.dma_start(tr[:, :, 512:D], sr[:, :, 512:D])
                    w_sb[name] = t

                load_w("q", wq, split_first=True)
                load_x("q", xq)
                nc.sync.dma_start(bq_sb[:], bqc[:])
                wqt = w_sb["q"][:].rearrange("p (c l) -> p c l", l=D)
                wqs = rw(wq)
                nc.sync.dma_start(wqt[:, :, 128:512], wqs[:, :, 128:512])
                nc.sync.dma_start(wqt[:, :, 512:D], wqs[:, :, 512:D])
                # consts
                nc.sync.dma_start(
                    wqa_sb[:].rearrange("p (c m) -> p c m", m=2 * H),
                    wqa[:].rearrange("(c p) m -> p c m", p=128),
                )
                for t, src in ((bk_sb, bkc), (bo_sb, boc),
                               (bqa_sb, bqac), (mk_sb, mkb), (ept_b, ept),
                               (ident, idn)):
                    nc.sync.dma_start(t[:], src[:])
                load_x("k", xk)
                load_w("k", wk)

                def proj_qk(xname, wname, dst, bcol):
                    xt = x_sb[xname][:].rearrange("p (c l) -> p c l", l=L)
                    wt = w_sb[wname][:].rearrange("p (c l) -> p c l", l=D)
                    for i in range(NT):
                        for lh in range(2):
                            ps = pool_ps.tile([128, 512], F32, tag="ps")
                            for c in range(NT):
                                nc.tensor.matmul(
                                    ps[:],
                                    wt[:, c, 128 * i : 128 * i + 128],
                                    xt[:, c, 512 * lh : 512 * lh + 512],
                                    start=(c == 0),
                                    stop=(c == NT - 1),
                                )
                            for half in range(2):
                                hh = 2 * i + half
                                nc.vector.tensor_scalar_add(
                                    dst[hh][0:64, 512 * lh : 512 * lh + 512],
                                    ps[64 * half : 64 * half + 64, :],
                                    bcol[64 * half : 64 * half + 64, i : i + 1],
                                )

                proj_qk("q", "q", qth, bq_sb)
                # augmented q rows for all heads: [32, 512] psum per half
                xt = x_sb["q"][:].rearrange("p (c l) -> p c l", l=L)
                wa = wqa_sb[:].rearrange("p (c m) -> p c m", m=2 * H)
                qaug = pp.tile([2 * H, L], BF16, tag="qaug")
                for lh in range(2):
                    ps2 = pool_ps2.tile([2 * H, 512], F32, tag="ps2")
                    for c in range(NT):
                        nc.tensor.matmul(
                            ps2[:], wa[:, c, :],
                            xt[:, c, 512 * lh : 512 * lh + 512],
                            start=(c == 0), stop=(c == NT - 1),
                        )
                    nc.vector.tensor_scalar_add(
                        qaug[:, 512 * lh : 512 * lh + 512], ps2[:], bqa_sb[:, 0:1]
                    )
                for h in range(H):
                    nc.gpsimd.dma_start(
                        qth[h][64:66, :], qaug[2 * h : 2 * h + 2, :]
                    )

                load_x("v", xv)
                load_w("v", wv)
                for h in range(2):
                    band_prep(h)
                proj_qk("k", "k", kth, bk_sb)

                # ---- V projection (natural layout, no bias) ----
                xt = x_sb["v"][:].rearrange("p (c l) -> p c l", l=L)
                wt = w_sb["v"][:].rearrange("p (c l) -> p c l", l=D)
                for dh_ in range(2):
                    for lt in range(NT):
                        ps = pool_ps.tile([128, 512], F32, tag="ps")
                        for c in range(NT):
                            nc.tensor.matmul(
                                ps[:],
                                xt[:, c, 128 * lt : 128 * lt + 128],
                                wt[:, c, 512 * dh_ : 512 * dh_ + 512],
                                start=(c == 0),
                                stop=(c == NT - 1),
                            )
                        nc.vector.tensor_copy(
                            v_sb[lt][:].rearrange("p (h c) -> p h c", c=65)[
                                :, 8 * dh_ : 8 * dh_ + 8, 0:64
                            ],
                            ps[:].rearrange("p (a b) -> p a b", a=8),
                        )

            # =========== attention ===========
            with (
                tc.tile_pool(name="attn", bufs=10) as pool_attn,
                tc.tile_pool(name="scratch", bufs=2) as pool_s,
                tc.tile_pool(name="psL", bufs=2, space="PSUM") as pool_psL,
                tc.tile_pool(name="psV", bufs=2, space="PSUM") as pool_psV,
            ):
                LOOK = 2
                opre = []
                for h in range(H):
                    q = qth[h]
                    k = kth[h]
                    gh = gtiles.pop(h)
                    gr = gh[:].rearrange("p (m j) -> p m j", j=BW)

                    attn = []
                    pavs = [pool_psV.tile([65, 512], F32, tag="pav",
                                           name=f"pav{lh}") for lh in range(2)]
                    for n in range(NT):
                        pl = pool_psL.tile([128, L], F32, tag="pl")
                        b0, b1 = max(n - 1, 0), min(n + 2, NT)
                        # far spans with clamp-tail aug rows (no g dependence)
                        spans = []
                        if 128 * (n + 2) < L:
                            spans.append((128 * (n + 2), L, 65))
                        if n - 1 > 0:
                            spans.append((0, 128 * (n - 1), 66))
                        for s0, s1, kk in spans:
                            c0 = s0
                            while c0 < s1:
                                c1 = min(s1, (c0 // 512 + 1) * 512)
                                nc.tensor.matmul(
                                    pl[:, c0:c1],
                                    k[0:kk, 128 * n : 128 * n + 128],
                                    q[0:kk, c0:c1],
                                    start=True, stop=True,
                                    skip_group_check=True,
                                )
                                c0 = c1
                        # band: QK (start) in <=512 chunks, then the
                        # transposed pos band accumulated via identity matmul
                        c0 = 128 * b0
                        while c0 < 128 * b1:
                            c1 = min(128 * b1, (c0 // 512 + 1) * 512)
                            nc.tensor.matmul(
                                pl[:, c0:c1],
                                k[0:64, 128 * n : 128 * n + 128],
                                q[0:64, c0:c1],
                                start=True, stop=False,
                                skip_group_check=True,
                            )
                            c0 = c1
                        for m in range(b0, b1):
                            jb = n - m + 1
                            nc.tensor.matmul(
                                pl[:, 128 * m : 128 * m + 128],
                                gr[:, m, 128 * jb : 128 * jb + 128],
                                ident[:],
                                start=False, stop=True,
                                skip_group_check=True,
                            )
                        at = pool_attn.tile([128, L], BF16, tag="at")
                        nc.scalar.activation(at[:], pl[:], EXP, bias=mk_sb[:, n : n + 1])
                        attn.append(at)
                        if h + LOOK < H:
                            band_prep_step(h + LOOK, n)

                    if h + LOOK < H:
                        band_finish(h + LOOK)
                    if h == 9:
                        nc.sync.dma_start(
                            wo_first[:].rearrange("p (c l) -> p c l", l=512),
                            rw(wo)[:, :, 0:512],
                        )
                    if h >= 14:
                        lh0 = h - 14
                        pso = pool_psP.tile([128, 512], F32, tag="pps",
                                            name=f"opre{lh0}")
                        wf0 = wo_first[:].rearrange("p (c l) -> p c l", l=512)
                        for c in range(NT - 1):
                            nc.tensor.matmul(
                                pso[:],
                                wf0[:, c, 0:128],
                                ct[c][:, 512 * lh0 : 512 * lh0 + 512],
                                start=(c == 0), stop=False,
                                skip_group_check=True,
                            )
                        opre.append(pso)
                    for lh in range(2):
                        for n in range(NT - 1):
                            nc.tensor.matmul(
                                pavs[lh][:],
                                v_sb[n][:, 65 * h : 65 * h + 65],
                                attn[n][:, 512 * lh : 512 * lh + 512],
                                start=(n == 0),
                                stop=False,
                            )
                    for lh in range(2):
                        nc.tensor.matmul(
                            pavs[lh][:],
                            v_sb[NT - 1][:, 65 * h : 65 * h + 65],
                            attn[NT - 1][:, 512 * lh : 512 * lh + 512],
                            start=False,
                            stop=True,
                        )
                    for lh in range(2):
                        pav = pavs[lh]
                        rec = pool_s.tile([1, 512], F32, tag="rec")
                        nc.vector.reciprocal(rec[:], pav[64:65, :])
                        pbm = pool_s.tile([64, 512], F32, tag="pbm")
                        nc.gpsimd.partition_broadcast(pbm[:], rec[:])
                        nc.vector.tensor_mul(
                            ct[h // 2][
                                64 * (h % 2) : 64 * (h % 2) + 64, 512 * lh : 512 * lh + 512
                            ],
                            pav[0:64, :],
                            pbm[:],
                        )

            # =========== output projection ===========
            with (
                tc.tile_pool(name="wout", bufs=1) as pool_wo,
                tc.tile_pool(name="oo", bufs=4) as pool_o,
                tc.tile_pool(name="ops", bufs=4, space="PSUM") as pool_ops,
            ):
                wot = pool_wo.tile([128, NT * D], BF16, tag="wo")
                wr = wot[:].rearrange("p (c l) -> p c l", l=D)
                wf = wo_first[:].rearrange("p (c l) -> p c l", l=512)
                sr = rw(wo)
                nc.sync.dma_start(wr[:, :, 512:768], sr[:, :, 512:768])
                nc.sync.dma_start(wr[:, :, 768:D], sr[:, :, 768:D])
                for i in range(NT):
                    ot = pool_o.tile([128, L], BF16, tag="ot")
                    for lh in range(2):
                        if i == 0:
                            ps = opre[lh]
                            nc.tensor.matmul(
                                ps[:],
                                wf[:, NT - 1, 0:128],
                                ct[NT - 1][:, 512 * lh : 512 * lh + 512],
                                start=False, stop=True,
                                skip_group_check=True,
                            )
                        else:
                            ps = pool_ops.tile([128, 512], F32, tag="ps")
                            wsrc, wcol = (wf, 128 * i) if i < 4 else (wr, 128 * i)
                            for c in range(NT):
                                nc.tensor.matmul(
                                    ps[:],
                                    wsrc[:, c, wcol : wcol + 128],
                                    ct[c][:, 512 * lh : 512 * lh + 512],
                                    start=(c == 0),
                                    stop=(c == NT - 1),
                                )
                        nc.vector.tensor_scalar_add(
                            ot[:, 512 * lh : 512 * lh + 512], ps[:], bo_sb[:, i : i + 1]
                        )
                    if i == NT - 1:
                        for lh in range(2):
                            nc.sync.dma_start(
                                outt[128 * i : 128 * i + 128, 512 * lh : 512 * lh + 512],
                                ot[:, 512 * lh : 512 * lh + 512],
                            )
                    else:
                        nc.sync.dma_start(outt[128 * i : 128 * i + 128, :], ot[:])

    nc.compile()
    return nc


def _get_nc():
    global _NC
    if _NC is None:
        _NC = _build()
    return _NC


def _prep_shared(Wq, bq, Wk, bk, Wv, bv, Wo, bo, pos_emb):
    bf = ml_dtypes.bfloat16
    Wq = np.asarray(Wq, np.float32)
    Wk = np.asarray(Wk, np.float32)
    Wv = np.asarray(Wv, np.float32)
    Wo = np.asarray(Wo, np.float32)
    bq = np.asarray(bq, np.float32)
    bk = np.asarray(bk, np.float32)
    bv = np.asarray(bv, np.float32)
    bo = np.asarray(bo, np.float32)
    ep = np.asarray(pos_emb, np.float32)

    wq_arr = np.ascontiguousarray(Wq.T / SCALE)
    wk_arr = np.ascontiguousarray(Wk.T)
    wv_arr = np.ascontiguousarray(Wv.T)
    wo_arr = np.ascontiguousarray(Wo.T)

    ep2 = np.stack([ep[0], ep[2 * 128] - ep[0]], axis=1)       # [64, 2]
    Wq3 = (Wq / SCALE).reshape(H, DH, D)
    # wqa[:, 2h+j] = sum_r ep2[r, j] * Wq3[h, r, :]
    wqa_arr = np.ascontiguousarray(
        np.einsum("rj,hrd->dhj", ep2, Wq3).reshape(D, 2 * H)
    )
    bq3 = (bq / SCALE).reshape(H, DH)
    bqa_arr = np.ascontiguousarray(np.einsum("rj,hr->hj", ep2, bq3).reshape(2 * H, 1))

    bo2 = bo + Wo @ bv
    return {
        "wq": wq_arr.astype(bf), "wk": wk_arr.astype(bf),
        "wv": wv_arr.astype(bf), "wo": wo_arr.astype(bf),
        "wqa": wqa_arr.astype(bf),
        "bqc": np.ascontiguousarray((bq / SCALE).reshape(NT, 128).T),
        "bkc": np.ascontiguousarray(bk.reshape(NT, 128).T),
        "boc": np.ascontiguousarray(bo2.reshape(NT, 128).T),
        "bqac": np.ascontiguousarray(bqa_arr),
        "ept": np.ascontiguousarray(ep.T).astype(bf),
        "idn": np.eye(128, dtype=np.float32).astype(bf),
    }


def _per_core_inputs(shared, inputs, b):
    bf = ml_dtypes.bfloat16
    mrow = np.asarray(inputs["mask"])[b].reshape(L).astype(bool)
    m = dict(shared)
    m["xq"] = np.ascontiguousarray(np.asarray(inputs["x_q"], np.float32)[b].T).astype(bf)
    m["xk"] = np.ascontiguousarray(np.asarray(inputs["x_k"], np.float32)[b].T).astype(bf)
    m["xv"] = np.ascontiguousarray(np.asarray(inputs["x_v"], np.float32)[b].T).astype(bf)
    m["mkb"] = np.ascontiguousarray(
        np.where(mrow, np.float32(-1e30), np.float32(0.0)).reshape(NT, 128).T
    )
    return m


def kernel(x_q, x_k, x_v, mask, Wq, bq, Wk, bk, Wv, bv, Wo, bo, pos_emb):
    nc = _get_nc()
    shared = _prep_shared(Wq, bq, Wk, bk, Wv, bv, Wo, bo, pos_emb)
    inputs = {"x_q": x_q, "x_k": x_k, "x_v": x_v, "mask": mask}
    in_maps = [_per_core_inputs(shared, inputs, b) for b in range(B)]
    res = run_bass_kernel_spmd(nc, in_maps, core_ids=list(range(B)))
    out = np.empty((B, L, D), np.float32)
    for b in range(B):
        out[b] = np.asarray(res.results[b]["outt"], np.float32).T
    return out
